# revision 1
# baseline (speedup 1.0000x reference)
"""Trainium2 Bass kernel for nn_ContextDrivingForce (dense MLP, 3 fused layers).

Math (per token row, D=896):
    u_proj = u @ W_a.T + b_a
    alpha  = sigmoid(sum(h * u_proj) / sqrt(D))
    u_att  = alpha * u
    g      = sigmoid([h, u_att] @ W_g.T + b_g)
    u_gate = g * u_att
    out    = gelu([h, u_gate, h*u_gate] @ W_f.T + b_f)        (exact erf gelu)

Distribution: data-parallel over the token axis across 8 NeuronCores,
weights replicated. All device tensors are feature-major ([D, tokens]);
the host transposes inputs/weights and the final output, so the device
performs no transposes at all.

Both sigmoids are computed as tanh ((sigmoid(x) = (tanh(x/2)+1)/2)) so that
every activation (tanh, gelu) lives in the single `gelu_and_others` ACT
table set -- no table reloads.  The 1/2 factors are folded into host-side
weight scaling:
    ua' := (tanh(logit/2)+1) * u          = 2*u_att     -> W_g[:,D:] *= 1/2
    ug' := (tanh(z2/2)+1) * ua'           = 4*u_gate    -> W_f[:,D:2D] *= 1/4
    hu' := h * ug'                        = 4*h*u_gate  -> W_f[:,2D:] *= 1/4
"""

import math
import sys
from contextlib import ExitStack

for _p in ("/root/.axon_site", "/root/.axon_site/_ro/trn_rl_repo"):
    if _p not in sys.path:
        sys.path.append(_p)

import ml_dtypes
import numpy as np

import concourse.bass as bass
import concourse.mybir as mybir
import concourse.tile as tile
from concourse import bacc
from concourse.bass_utils import run_bass_kernel_spmd

P = 128
D = 896
KD = D // P  # 7 feature tiles
N_TOK = 16384
N_CORES = 8
NPC = N_TOK // N_CORES  # 2048 tokens per core

F32 = mybir.dt.float32
AF = mybir.ActivationFunctionType
ALU = mybir.AluOpType


def build_nc(npc=NPC, T=512, mode="bf16", mm_bufs=5, act_bufs=None, gelu_native=True):
    if act_bufs is None:
        act_bufs = 2 if mode == "bf16" else 1
    """Build the single-core Bass program (same program runs SPMD on all cores)."""
    if mode == "bf16":
        cdt = mybir.dt.bfloat16
        mdt = mybir.dt.bfloat16
    elif mode == "fp32r":
        cdt = F32
        mdt = mybir.dt.float32r
    elif mode == "fp32":
        cdt = F32
        mdt = F32
    else:
        raise ValueError(mode)

    n_chunks = npc // T
    assert n_chunks * T == npc

    nc = bacc.Bacc()
    # inputs are chunk-major [P, n_chunks, KD, T]: each chunk DMA reads
    # 7KB contiguous per partition (near-peak DMA efficiency)
    hT_d = nc.declare_dram_parameter("hT", [P, npc // T, KD, T], cdt, isOutput=False)
    uT_d = nc.declare_dram_parameter("uT", [P, npc // T, KD, T], cdt, isOutput=False)
    # weights as three DMAs, W_a first, so layer-1 matmuls can start while
    # W_g / W_f are still in flight (HWDGE ring is FIFO in trigger order)
    wa_d = nc.declare_dram_parameter("wa", [P, KD, D], cdt, isOutput=False)
    wg_d = nc.declare_dram_parameter("wg", [P, 2 * KD, D], cdt, isOutput=False)
    wf_d = nc.declare_dram_parameter("wf", [P, 3 * KD, D], cdt, isOutput=False)
    bias_d = nc.declare_dram_parameter("biasp", [P, 3 * KD], F32, isOutput=False)
    gT_d = nc.declare_dram_parameter("gT", [D, npc], F32, isOutput=True)

    inv_sqrt_d = 1.0 / math.sqrt(D)

    def mm(ps, lhsT, rhs, start, stop):
        if mdt != cdt:
            lhsT = lhsT.bitcast(mdt)
            rhs = rhs.bitcast(mdt)
        nc.tensor.matmul(ps, lhsT=lhsT, rhs=rhs, start=start, stop=stop)

    with tile.TileContext(nc) as tc, ExitStack() as ctx:
        wp = ctx.enter_context(tc.tile_pool(name="weights", bufs=1))
        hp = ctx.enter_context(tc.tile_pool(name="hp", bufs=act_bufs))
        up = ctx.enter_context(tc.tile_pool(name="up", bufs=act_bufs))
        uap = ctx.enter_context(tc.tile_pool(name="uap", bufs=act_bufs))
        ugp = ctx.enter_context(tc.tile_pool(name="ugp", bufs=act_bufs))
        hup = ctx.enter_context(tc.tile_pool(name="hup", bufs=act_bufs))
        sp = ctx.enter_context(tc.tile_pool(name="small", bufs=3))
        op = ctx.enter_context(tc.tile_pool(name="outp", bufs=3))
        pp = ctx.enter_context(tc.tile_pool(name="psum", bufs=1, space="PSUM"))

        bias_sb = wp.tile([P, 3 * KD], F32, name="biasp")
        nc.sync.dma_start(bias_sb, bias_d[:, :])
        ones_col = wp.tile([P, 1], cdt, name="ones_col")
        nc.vector.memset(ones_col, 1.0)
        ones_row = wp.tile([1, P], cdt, name="ones_row")
        nc.vector.memset(ones_row, 1.0)

        def load_chunk(c):
            h_sb = hp.tile([P, KD, T], cdt, name=f"h{c}", tag="h")
            nc.sync.dma_start(h_sb, hT_d[:, c])
            u_sb = up.tile([P, KD, T], cdt, name=f"u{c}", tag="u")
            nc.sync.dma_start(u_sb, uT_d[:, c])
            return h_sb, u_sb

        # Prelude: stream W_a + chunk-0 inputs per k-tile, interleaved in DMA
        # FIFO order, so layer-1 matmul k=0 can start after ~2 small DMAs
        # instead of waiting for all prelude bytes.
        wa_sb = wp.tile([P, KD, D], cdt, name="wa")
        h0_sb = hp.tile([P, KD, T], cdt, name="h0", tag="h")
        u0_sb = up.tile([P, KD, T], cdt, name="u0", tag="u")
        for k in range(KD):
            nc.sync.dma_start(wa_sb[:, k], wa_d[:, k])
            nc.sync.dma_start(u0_sb[:, k], uT_d[:, 0, k])
            nc.sync.dma_start(h0_sb[:, k], hT_d[:, 0, k])
        chunk0 = (h0_sb, u0_sb)
        # W_g / W_f staged to match consumption order (h-side k-tiles first)
        wg_sb = wp.tile([P, 2 * KD, D], cdt, name="wg")
        nc.sync.dma_start(wg_sb[:, :KD], wg_d[:, :KD])
        nc.sync.dma_start(wg_sb[:, KD:], wg_d[:, KD:])
        wf_sb = wp.tile([P, 3 * KD, D], cdt, name="wf")
        for j in range(3):
            nc.sync.dma_start(wf_sb[:, j * KD:(j + 1) * KD],
                              wf_d[:, j * KD:(j + 1) * KD])

        M_GROUPS = [list(range(0, 4)), list(range(4, KD))]

        for c in range(n_chunks):
            cs = bass.ds(c * T, T)
            h_sb, u_sb = chunk0 if c == 0 else load_chunk(c)

            # ---- layer 1: u_proj = u @ W_a.T (feature-major), fused logit
            # reduce. Grouped-k-major: the k-loop is innermost across a group
            # of <=4 M-tiles so compute starts as soon as k-tile 0 arrives.
            red = pp.tile([1, T], F32, name=f"red{c}", tag="red", bufs=1)
            tmps = []
            for grp in M_GROUPS:
                pss = {m: pp.tile([P, T], F32, name=f"ps1_{c}_{m}", tag="mm",
                                  bufs=mm_bufs) for m in grp}
                for k in range(KD):
                    for m in grp:
                        mm(pss[m], wa_sb[:, k, m * P:(m + 1) * P], u_sb[:, k, :],
                           start=(k == 0), stop=(k == KD - 1))
                for m in grp:
                    # tmp = (u_proj + b_a) * h   (one fused DVE op)
                    tmp = sp.tile([P, T], cdt, name=f"tmp{c}_{m}", tag="tmp",
                                  bufs=KD)
                    nc.vector.scalar_tensor_tensor(
                        out=tmp, in0=pss[m], scalar=bias_sb[:, m:m + 1],
                        in1=h_sb[:, m, :], op0=ALU.add, op1=ALU.mult)
                    tmps.append(tmp)
            # partition-reduce the 7 tmp tiles into the logit row
            for m in range(KD):
                mm(red, ones_col, tmps[m], start=(m == 0), stop=(m == KD - 1))

            # alpha' = tanh(logit / (2 sqrt(D))) = 2*sigmoid(logit) - 1
            alpha = sp.tile([1, T], cdt, name=f"al{c}", tag="alpha", bufs=2)
            nc.scalar.activation(alpha, red, AF.Tanh, scale=inv_sqrt_d * 0.5)
            # broadcast across partitions with a rank-1 matmul
            ab = pp.tile([P, T], F32, name=f"ab{c}", tag="ab", bufs=2)
            mm(ab, ones_row, alpha, start=True, stop=True)

            # ua' = (alpha'+1) * u = 2 * u_att
            ua_sb = uap.tile([P, KD, T], cdt, name=f"ua{c}", tag="ua")
            for k in range(KD):
                nc.vector.scalar_tensor_tensor(
                    out=ua_sb[:, k, :], in0=ab, scalar=1.0, in1=u_sb[:, k, :],
                    op0=ALU.add, op1=ALU.mult)

            # ---- layer 2: z2 = [h, u_att] @ W_g.T ; ug' = (tanh((z2+b)/2)+1)*ua'
            ug_sb = ugp.tile([P, KD, T], cdt, name=f"ug{c}", tag="ug")
            for grp in M_GROUPS:
                pss = {m: pp.tile([P, T], F32, name=f"ps2_{c}_{m}", tag="mm",
                                  bufs=mm_bufs) for m in grp}
                for k in range(2 * KD):
                    rhs = h_sb[:, k, :] if k < KD else ua_sb[:, k - KD, :]
                    for m in grp:
                        mm(pss[m], wg_sb[:, k, m * P:(m + 1) * P], rhs,
                           start=(k == 0), stop=(k == 2 * KD - 1))
                for m in grp:
                    t2 = sp.tile([P, T], cdt, name=f"t2_{c}_{m}", tag="t2")
                    nc.scalar.activation(t2, pss[m], AF.Tanh,
                                         bias=bias_sb[:, KD + m:KD + m + 1],
                                         scale=0.5)
                    nc.vector.scalar_tensor_tensor(
                        out=ug_sb[:, m, :], in0=t2, scalar=1.0,
                        in1=ua_sb[:, m, :], op0=ALU.add, op1=ALU.mult)

            # hu' = h * ug' (= 4*h*u_gate; the 1/4 is folded into W_f cols)
            hu_sb = hup.tile([P, KD, T], cdt, name=f"hu{c}", tag="hu")
            for k in range(KD):
                nc.vector.tensor_mul(out=hu_sb[:, k, :], in0=h_sb[:, k, :],
                                     in1=ug_sb[:, k, :])

            # ---- layer 3: out = gelu([h, ug', hu'] @ W_f'.T + b_f)
            for m in range(KD):
                ps = pp.tile([P, T], F32, name=f"ps3_{c}_{m}", tag="mm", bufs=mm_bufs)
                for k in range(3 * KD):
                    if k < KD:
                        rhs = h_sb[:, k, :]
                    elif k < 2 * KD:
                        rhs = ug_sb[:, k - KD, :]
                    else:
                        rhs = hu_sb[:, k - 2 * KD, :]
                    mm(ps, wf_sb[:, k, m * P:(m + 1) * P], rhs,
                       start=(k == 0), stop=(k == 3 * KD - 1))
                outp = op.tile([P, T], F32, name=f"o{c}_{m}", tag="out")
                nc.scalar.activation(outp, ps,
                                     AF.Gelu if gelu_native else AF.Identity,
                                     bias=bias_sb[:, 2 * KD + m:2 * KD + m + 1],
                                     scale=1.0)
                # output stores ride the ACT HWDGE ring so they never block
                # the input-load FIFO on the SP ring
                nc.scalar.dma_start(gT_d[m * P:(m + 1) * P, cs], outp)
    nc.compile()  # bacc passes: split >1-wait instrs onto EventSemaphores, etc.
    return nc


def prep_inputs(h_t, u_t, W_a_w, W_a_b, W_g_w, W_g_b, W_f_w, W_f_b,
                npc=NPC, T=512, mode="bf16"):
    """Host-side layout prep: transpose to feature-major, fold tanh-trick
    scales into the weights, pack per-out-feature biases, shard tokens."""
    np_dt = ml_dtypes.bfloat16 if mode == "bf16" else np.float32

    h = np.asarray(h_t, np.float32)
    u = np.asarray(u_t, np.float32)
    Wa = np.asarray(W_a_w, np.float32)
    Wg = np.asarray(W_g_w, np.float32)
    Wf = np.asarray(W_f_w, np.float32)
    ba = np.asarray(W_a_b, np.float32)
    bg = np.asarray(W_g_b, np.float32)
    bf = np.asarray(W_f_b, np.float32)

    waT = Wa.T  # [in, out]
    wgT = np.concatenate([Wg[:, :D], Wg[:, D:] * 0.5], axis=1).T
    wfT = np.concatenate([Wf[:, :D], Wf[:, D:2 * D] * 0.25, Wf[:, 2 * D:] * 0.25],
                         axis=1).T

    def wpack(w):  # [K_in, D_out] -> [128, K_in/128, D_out]
        return np.ascontiguousarray(
            w.reshape(-1, P, D).transpose(1, 0, 2)).astype(np_dt)

    wa_p, wg_p, wf_p = wpack(waT), wpack(wgT), wpack(wfT)
    # bias pack: [128, 21] fp32; column m is out-features [m*128,(m+1)*128)
    # of b_a (layer1), 0.5*b_g (layer2 tanh arg), b_f (layer3)
    biasp = np.ascontiguousarray(
        np.concatenate([ba, 0.5 * bg, bf]).reshape(3 * KD, P).T).astype(np.float32)

    # chunk-major input pack: [P, n_chunks, KD, T] per core, so each chunk's
    # DMA is 7KB-contiguous per partition. From token-major [N, D]:
    # pack[p, c, k, t] = x[core*npc + c*T + t, k*128 + p]
    nch = npc // T

    def xpack(x, i):  # x [N, D] -> [P, nch, KD, T] for core i
        blk = x[i * npc:(i + 1) * npc]                    # [npc, D]
        blk = blk.reshape(nch, T, KD, P)                  # [c, t, k, p]
        return np.ascontiguousarray(
            blk.transpose(3, 0, 2, 1)).astype(np_dt)      # [p, c, k, t]

    n_cores = h.shape[0] // npc
    in_maps = []
    for i in range(n_cores):
        in_maps.append({
            "hT": xpack(h, i),
            "uT": xpack(u, i),
            "wa": wa_p, "wg": wg_p, "wf": wf_p, "biasp": biasp,
        })
    return in_maps


_NC_CACHE = {}


def _get_nc(npc=NPC, T=512, mode="bf16"):
    key = (npc, T, mode)
    if key not in _NC_CACHE:
        _NC_CACHE[key] = build_nc(npc=npc, T=T, mode=mode)
    return _NC_CACHE[key]


def run(inputs, npc=NPC, T=None, mode="bf16", trace=False, **kw):
    """Run the SPMD kernel; returns (full_output [N,D] fp32, BassKernelResults)."""
    if T is None:
        T = 512 if mode == "bf16" else 256
    nc = _get_nc(npc=npc, T=T, mode=mode)
    in_maps = prep_inputs(
        inputs["h_t"], inputs["u_t"], inputs["W_a_w"], inputs["W_a_b"],
        inputs["W_g_w"], inputs["W_g_b"], inputs["W_f_w"], inputs["W_f_b"],
        npc=npc, T=T, mode=mode)
    res = run_bass_kernel_spmd(nc, in_maps, list(range(len(in_maps))),
                               trace=trace, **kw)
    out = np.concatenate(
        [np.asarray(r["gT"], np.float32).T for r in res.results], axis=0)
    return out, res


def kernel(h_t, u_t, token_idx, u_all, W_a_w, W_a_b, W_g_w, W_g_b, W_f_w, W_f_b):
    # token_idx / u_all are unused by the reference math.
    inputs = {"h_t": h_t, "u_t": u_t, "W_a_w": W_a_w, "W_a_b": W_a_b,
              "W_g_w": W_g_w, "W_g_b": W_g_b, "W_f_w": W_f_w, "W_f_b": W_f_b}
    out, _ = run(inputs)
    return out


if __name__ == "__main__":
    # tiny smoke test through CoreSim is in test.py; direct run does HW.
    rng = np.random.default_rng(0)
    fake = {
        "h_t": rng.standard_normal((N_TOK, D), dtype=np.float32),
        "u_t": rng.standard_normal((N_TOK, D), dtype=np.float32),
        "W_a_w": rng.standard_normal((D, D), dtype=np.float32) * 0.02,
        "W_a_b": rng.standard_normal((D,), dtype=np.float32) * 0.02,
        "W_g_w": rng.standard_normal((D, 2 * D), dtype=np.float32) * 0.02,
        "W_g_b": rng.standard_normal((D,), dtype=np.float32) * 0.02,
        "W_f_w": rng.standard_normal((D, 3 * D), dtype=np.float32) * 0.02,
        "W_f_b": rng.standard_normal((D,), dtype=np.float32) * 0.02,
    }
    out, res = run(fake)
    print("out", out.shape, out.dtype, "exec_time_ns", res.exec_time_ns)



# revision 3
# speedup vs baseline: 1.2968x; 1.2968x over previous
"""Trainium2 Bass kernel for nn_ContextDrivingForce (dense MLP, 3 fused layers).

Math (per token row, D=896):
    u_proj = u @ W_a.T + b_a
    alpha  = sigmoid(sum(h * u_proj) / sqrt(D))
    u_att  = alpha * u
    g      = sigmoid([h, u_att] @ W_g.T + b_g)
    u_gate = g * u_att
    out    = gelu([h, u_gate, h*u_gate] @ W_f.T + b_f)        (exact erf gelu)

Distribution: data-parallel over tokens across 8 NeuronCores, weights
replicated. Device tensors feature-major ([D, tokens]); host transposes.

Precision strategy (validated by host-side elementwise simulation of the
exact device dataflow, rel err vs fp32 reference):
  - Layers 1+2 matmuls in fp8e4m3 with DoubleRow (2 k-tiles per MM issue).
    Their output error is damped by the sigmoid gates (|sigma'| <= 1/4).
  - Layer 3: the h-part (87% of z3 variance) stays bf16; the ug/hu parts
    optionally fp8+DoubleRow ("cand2").
  - Elementwise (DVE) chain uses clean bf16 copies of u/h/ua/ug so fp8
    quantization error does not compound through the products.
  - Activations carry embedded scale S_ACT=4, weights S_W=64 (fp8 needs
    operands in normal range; all folds are powers of two).

Sigmoids via tanh (sigmoid(x) = (tanh(x/2)+1)/2) so every ACT op uses the
single `gelu_and_others` table set. Factors of 2 folded into host-side
weight scaling:
    ua' := (tanh(logit/2)+1) * u = 2*u_att   -> W_g ua-cols *= 1/2
    ug' := (t2+1) * ua'          = 4*u_gate  -> W_f ug-cols *= 1/4
    hu' := h * ug'               = 4*h*u_gate-> W_f hu-cols *= 1/4
"""

import math
import sys
from contextlib import ExitStack

for _p in ("/root/.axon_site", "/root/.axon_site/_ro/trn_rl_repo"):
    if _p not in sys.path:
        sys.path.append(_p)

import ml_dtypes
import numpy as np

import concourse.bass as bass
import concourse.mybir as mybir
import concourse.tile as tile
from concourse import bacc
from concourse.bass_utils import run_bass_kernel_spmd

P = 128
D = 896
KD = D // P  # 7 feature tiles
N_TOK = 16384
N_CORES = 8
NPC = N_TOK // N_CORES  # 2048 tokens per core

F32 = mybir.dt.float32
BF16 = mybir.dt.bfloat16
F8 = mybir.dt.float8e4
AF = mybir.ActivationFunctionType
ALU = mybir.AluOpType
DR = mybir.MatmulPerfMode.DoubleRow

S_ACT = 4.0     # embedded scale on h/u and all derived activations
S_W = 64.0      # embedded scale on all weights
S1 = S_ACT * S_W


def build_nc(npc=NPC, T=512, l3fp8=True, clean_hu=True, mm_bufs=5, act_bufs=2):
    """Single-core Bass program (runs SPMD on all 8 cores).

    l3fp8: layer-3 ug/hu k-tiles in fp8 DoubleRow (cand2) vs bf16 (cand1).
    clean_hu: hu computed from a bf16 ug copy instead of the fp8 ug tile.
    """
    n_chunks = npc // T
    assert n_chunks * T == npc

    nc = bacc.Bacc()
    # bf16 h/u glued: [P, nch, 2(h|u), KD, T]
    hub_d = nc.declare_dram_parameter("hub", [P, n_chunks, 2, KD, T], BF16,
                                      isOutput=False)
    h8_d = nc.declare_dram_parameter("h8", [P, n_chunks, KD, T], F8, isOutput=False)
    u8_d = nc.declare_dram_parameter("u8", [P, n_chunks, KD, T], F8, isOutput=False)
    wa_d = nc.declare_dram_parameter("wa", [P, KD, D], F8, isOutput=False)
    # wg k-order: [ua-cols (folded 0.5) | h-cols] to match the uah8 tile
    wg_d = nc.declare_dram_parameter("wg", [P, 2 * KD, D], F8, isOutput=False)
    wfh_d = nc.declare_dram_parameter("wfh", [P, KD, D], BF16, isOutput=False)
    l3dt = F8 if l3fp8 else BF16
    wfuh_d = nc.declare_dram_parameter("wfuh", [P, 2 * KD, D], l3dt, isOutput=False)
    bias_d = nc.declare_dram_parameter("biasp", [P, 3 * KD], F32, isOutput=False)
    gT_d = nc.declare_dram_parameter("gT", [D, npc], BF16, isOutput=True)

    inv_sqrt_d = 1.0 / math.sqrt(D)

    with tile.TileContext(nc) as tc, ExitStack() as ctx:
        wp = ctx.enter_context(tc.tile_pool(name="weights", bufs=1))
        hubp = ctx.enter_context(tc.tile_pool(name="hubp", bufs=act_bufs))
        up = ctx.enter_context(tc.tile_pool(name="up", bufs=act_bufs))
        uahp = ctx.enter_context(tc.tile_pool(name="uahp", bufs=act_bufs))
        uabp = ctx.enter_context(tc.tile_pool(name="uabp", bufs=act_bufs))
        ughup = ctx.enter_context(tc.tile_pool(name="ughup", bufs=act_bufs))
        ugbp = ctx.enter_context(tc.tile_pool(name="ugbp", bufs=act_bufs))
        tmpp = ctx.enter_context(tc.tile_pool(name="tmpp", bufs=act_bufs))
        sp = ctx.enter_context(tc.tile_pool(name="small", bufs=3))
        op = ctx.enter_context(tc.tile_pool(name="outp", bufs=3))
        pp = ctx.enter_context(tc.tile_pool(name="psum", bufs=1, space="PSUM"))

        bias_sb = wp.tile([P, 3 * KD], F32, name="biasp")
        nc.sync.dma_start(bias_sb, bias_d[:, :])
        ones_col = wp.tile([P, 1], BF16, name="ones_col")
        nc.vector.memset(ones_col, 1.0)
        ones_row = wp.tile([1, P], BF16, name="ones_row")
        nc.vector.memset(ones_row, 1.0)

        # ---- weight + chunk-0 prelude, staged for fast compute start ----
        wa_sb = wp.tile([P, KD, D], F8, name="wa")
        u8_0 = up.tile([P, KD, T], F8, name="u8_0", tag="u8")
        uah_0 = uahp.tile([P, 2 * KD, T], F8, name="uah0", tag="uah")
        # first DR pair's operands first: L1 can start after ~0.4 MB
        nc.sync.dma_start(wa_sb[:, 0:2], wa_d[:, 0:2])
        nc.sync.dma_start(u8_0[:, 0:2], u8_d[:, 0, 0:2])
        nc.sync.dma_start(wa_sb[:, 2:], wa_d[:, 2:])
        nc.sync.dma_start(u8_0[:, 2:], u8_d[:, 0, 2:])
        nc.sync.dma_start(uah_0[:, KD:], h8_d[:, 0])  # h8 -> uah slots 7..13
        wg_sb = wp.tile([P, 2 * KD, D], F8, name="wg")
        nc.sync.dma_start(wg_sb, wg_d[:, :])
        wfh_sb = wp.tile([P, KD, D], BF16, name="wfh")
        nc.sync.dma_start(wfh_sb, wfh_d[:, :])
        wfuh_sb = wp.tile([P, 2 * KD, D], l3dt, name="wfuh")
        nc.sync.dma_start(wfuh_sb, wfuh_d[:, :])
        # bf16 h|u rides the GPSIMD ring so it never blocks the fp8 stream;
        # split so the first h k-tiles land early for the tmp STTs
        hub_0 = hubp.tile([P, 2, KD, T], BF16, name="hub0", tag="hub")
        nc.gpsimd.dma_start(hub_0[:, 0, :4], hub_d[:, 0, 0, :4])
        nc.gpsimd.dma_start(hub_0[:, 0, 4:], hub_d[:, 0, 0, 4:])
        nc.gpsimd.dma_start(hub_0[:, 1], hub_d[:, 0, 1])
        chunk0 = (hub_0, u8_0, uah_0)

        def load_chunk(c):
            hub = hubp.tile([P, 2, KD, T], BF16, name=f"hub{c}", tag="hub")
            nc.gpsimd.dma_start(hub, hub_d[:, c])
            u8 = up.tile([P, KD, T], F8, name=f"u8_{c}", tag="u8")
            nc.sync.dma_start(u8, u8_d[:, c])
            uah = uahp.tile([P, 2 * KD, T], F8, name=f"uah{c}", tag="uah")
            nc.sync.dma_start(uah[:, KD:], h8_d[:, c])
            return hub, u8, uah

        M_GROUPS = [list(range(0, 4)), list(range(4, KD))]

        for c in range(n_chunks):
            cs = bass.ds(c * T, T)
            hub, u8, uah = chunk0 if c == 0 else load_chunk(c)
            h_bf = hub[:, 0]   # [P, KD, T] bf16, = S_ACT*h
            u_bf = hub[:, 1]

            # ---- layer 1: psum1 = (S1) * u_proj ; fused logit reduce
            tmp = tmpp.tile([P, KD, T], BF16, name=f"tmp{c}", tag="tmp")
            for grp in M_GROUPS:
                pss = {m: pp.tile([P, T], F32, name=f"ps1_{c}_{m}", tag="mm",
                                  bufs=mm_bufs) for m in grp}
                for k in range(0, KD - 1, 2):     # 3 DoubleRow pairs
                    for m in grp:
                        nc.tensor.matmul(pss[m], lhsT=wa_sb[:, k:k + 2, m * P:(m + 1) * P],
                                         rhs=u8[:, k:k + 2, :], start=(k == 0),
                                         stop=False, perf_mode=DR)
                for m in grp:                     # odd k-tile 6, plain fp8
                    nc.tensor.matmul(pss[m], lhsT=wa_sb[:, KD - 1, m * P:(m + 1) * P],
                                     rhs=u8[:, KD - 1, :], start=False, stop=True)
                for m in grp:
                    # tmp = (psum1 + S1*b_a) * (S_ACT*h)   (one fused DVE op)
                    nc.vector.scalar_tensor_tensor(
                        out=tmp[:, m, :], in0=pss[m], scalar=bias_sb[:, m:m + 1],
                        in1=h_bf[:, m, :], op0=ALU.add, op1=ALU.mult)
            red = pp.tile([1, T], F32, name=f"red{c}", tag="red", bufs=1)
            for m in range(KD):
                nc.tensor.matmul(red, lhsT=ones_col, rhs=tmp[:, m, :],
                                 start=(m == 0), stop=(m == KD - 1))

            # alpha' = tanh(logit/2) = 2*sigmoid(logit)-1; logit scale S1*S_ACT
            alpha = sp.tile([1, T], BF16, name=f"al{c}", tag="alpha", bufs=2)
            nc.scalar.activation(alpha, red, AF.Tanh,
                                 scale=inv_sqrt_d * 0.5 / (S1 * S_ACT))
            ab = pp.tile([P, T], F32, name=f"ab{c}", tag="ab", bufs=2)
            nc.tensor.matmul(ab, lhsT=ones_row, rhs=alpha, start=True, stop=True)

            # ua' = (alpha'+1)*u: fp8 copy (L2 rhs) + clean bf16 copy (DVE)
            ua_bf = uabp.tile([P, KD, T], BF16, name=f"uab{c}", tag="uab")
            for k in range(KD):
                nc.vector.scalar_tensor_tensor(
                    out=uah[:, k, :], in0=ab, scalar=1.0, in1=u8[:, k, :],
                    op0=ALU.add, op1=ALU.mult)
                nc.vector.scalar_tensor_tensor(
                    out=ua_bf[:, k, :], in0=ab, scalar=1.0, in1=u_bf[:, k, :],
                    op0=ALU.add, op1=ALU.mult)

            # ---- layer 2: z2 = [ua', h] @ wg (7 DR pairs over uah)
            ughu = ughup.tile([P, 2 * KD, T], l3dt, name=f"ughu{c}", tag="ughu")
            ug_bf = ugbp.tile([P, KD, T], BF16, name=f"ugb{c}", tag="ugb") \
                if (l3fp8 and clean_hu) else None
            for grp in M_GROUPS:
                pss = {m: pp.tile([P, T], F32, name=f"ps2_{c}_{m}", tag="mm",
                                  bufs=mm_bufs) for m in grp}
                for k in range(0, 2 * KD, 2):
                    for m in grp:
                        nc.tensor.matmul(pss[m], lhsT=wg_sb[:, k:k + 2, m * P:(m + 1) * P],
                                         rhs=uah[:, k:k + 2, :], start=(k == 0),
                                         stop=(k == 2 * KD - 2), perf_mode=DR)
                for m in grp:
                    t2 = sp.tile([P, T], BF16, name=f"t2_{c}_{m}", tag="t2")
                    nc.scalar.activation(t2, pss[m], AF.Tanh,
                                         bias=bias_sb[:, KD + m:KD + m + 1],
                                         scale=0.5 / S1)
                    # ug' = (t2+1)*ua'  (fp8 for L3 rhs; bf16 copy for hu)
                    nc.vector.scalar_tensor_tensor(
                        out=ughu[:, m, :], in0=t2, scalar=1.0,
                        in1=ua_bf[:, m, :], op0=ALU.add, op1=ALU.mult)
                    if ug_bf is not None:
                        nc.vector.scalar_tensor_tensor(
                            out=ug_bf[:, m, :], in0=t2, scalar=1.0,
                            in1=ua_bf[:, m, :], op0=ALU.add, op1=ALU.mult)
                    # hu' = h*ug' = (h_bf * 1/S_ACT) * ug
                    ug_src = ug_bf if ug_bf is not None else ughu
                    nc.vector.scalar_tensor_tensor(
                        out=ughu[:, KD + m, :], in0=h_bf[:, m, :],
                        scalar=1.0 / S_ACT, in1=ug_src[:, m, :],
                        op0=ALU.mult, op1=ALU.mult)

            # ---- layer 3: out = gelu([h]bf16 + [ug', hu']fp8-DR + b_f)
            for m in range(KD):
                ps = pp.tile([P, T], F32, name=f"ps3_{c}_{m}", tag="mm", bufs=mm_bufs)
                for k in range(KD):
                    nc.tensor.matmul(ps, lhsT=wfh_sb[:, k, m * P:(m + 1) * P],
                                     rhs=h_bf[:, k, :], start=(k == 0), stop=False)
                if l3fp8:
                    for k in range(0, 2 * KD, 2):
                        nc.tensor.matmul(ps, lhsT=wfuh_sb[:, k:k + 2, m * P:(m + 1) * P],
                                         rhs=ughu[:, k:k + 2, :], start=False,
                                         stop=(k == 2 * KD - 2), perf_mode=DR)
                else:
                    for k in range(2 * KD):
                        nc.tensor.matmul(ps, lhsT=wfuh_sb[:, k, m * P:(m + 1) * P],
                                         rhs=ughu[:, k, :], start=False,
                                         stop=(k == 2 * KD - 1))
                outp = op.tile([P, T], BF16, name=f"o{c}_{m}", tag="out")
                nc.scalar.activation(outp, ps, AF.Gelu,
                                     bias=bias_sb[:, 2 * KD + m:2 * KD + m + 1],
                                     scale=1.0 / S1)
                # output stores ride the ACT HWDGE ring
                nc.scalar.dma_start(gT_d[m * P:(m + 1) * P, cs], outp)
    nc.compile()
    return nc


def prep_inputs(h_t, u_t, W_a_w, W_a_b, W_g_w, W_g_b, W_f_w, W_f_b,
                npc=NPC, T=512, l3fp8=True):
    """Host-side: transpose to feature-major, fold scales, quantize, shard."""
    f8 = ml_dtypes.float8_e4m3
    bf16 = ml_dtypes.bfloat16

    h = np.asarray(h_t, np.float32)
    u = np.asarray(u_t, np.float32)
    Wa = np.asarray(W_a_w, np.float32)
    Wg = np.asarray(W_g_w, np.float32)
    Wf = np.asarray(W_f_w, np.float32)
    ba = np.asarray(W_a_b, np.float32)
    bg = np.asarray(W_g_b, np.float32)
    bf = np.asarray(W_f_b, np.float32)

    waT = S_W * Wa.T
    # k-order [ua-cols (x0.5) | h-cols] to match the uah8 tile layout
    wgT = S_W * np.concatenate([Wg[:, D:] * 0.5, Wg[:, :D]], axis=1).T
    wfhT = S_W * Wf[:, :D].T
    wfuhT = S_W * np.concatenate([Wf[:, D:2 * D] * 0.25, Wf[:, 2 * D:] * 0.25],
                                 axis=1).T

    def wpack(w, dt):  # [K_in, D_out] -> [128, K_in/128, D_out]
        return np.ascontiguousarray(
            w.reshape(-1, P, D).transpose(1, 0, 2)).astype(dt)

    wa_p = wpack(waT, f8)
    wg_p = wpack(wgT, f8)
    wfh_p = wpack(wfhT, bf16)
    wfuh_p = wpack(wfuhT, f8 if l3fp8 else bf16)
    biasp = np.ascontiguousarray(
        np.concatenate([S1 * ba, 0.5 * bg, bf]).reshape(3 * KD, P).T
    ).astype(np.float32)

    nch = npc // T

    def xpack(x, i, dt):  # x [N, D] -> [P, nch, KD, T] for core i
        blk = x[i * npc:(i + 1) * npc]                    # [npc, D]
        blk = blk.reshape(nch, T, KD, P)                  # [c, t, k, p]
        return np.ascontiguousarray(blk.transpose(3, 0, 2, 1)).astype(dt)

    hs = S_ACT * h
    us = S_ACT * u
    n_cores = h.shape[0] // npc
    in_maps = []
    for i in range(n_cores):
        hp8 = xpack(hs, i, f8)
        up8 = xpack(us, i, f8)
        hub = np.stack([xpack(hs, i, bf16), xpack(us, i, bf16)], axis=2)
        # hub shape [P, nch, 2, KD, T]
        in_maps.append({
            "hub": np.ascontiguousarray(hub),
            "h8": hp8, "u8": up8,
            "wa": wa_p, "wg": wg_p, "wfh": wfh_p, "wfuh": wfuh_p,
            "biasp": biasp,
        })
    return in_maps


_NC_CACHE = {}


def _get_nc(npc=NPC, T=512, l3fp8=True, clean_hu=True):
    key = (npc, T, l3fp8, clean_hu)
    if key not in _NC_CACHE:
        _NC_CACHE[key] = build_nc(npc=npc, T=T, l3fp8=l3fp8, clean_hu=clean_hu)
    return _NC_CACHE[key]


def run(inputs, npc=NPC, T=512, l3fp8=True, clean_hu=True, trace=False, **kw):
    """Run the SPMD kernel; returns (full fp32 [N,D] output, BassKernelResults)."""
    nc = _get_nc(npc=npc, T=T, l3fp8=l3fp8, clean_hu=clean_hu)
    in_maps = prep_inputs(
        inputs["h_t"], inputs["u_t"], inputs["W_a_w"], inputs["W_a_b"],
        inputs["W_g_w"], inputs["W_g_b"], inputs["W_f_w"], inputs["W_f_b"],
        npc=npc, T=T, l3fp8=l3fp8)
    res = run_bass_kernel_spmd(nc, in_maps, list(range(len(in_maps))),
                               trace=trace, **kw)
    out = np.concatenate(
        [np.asarray(r["gT"]).astype(np.float32).T for r in res.results], axis=0)
    return out, res


def kernel(h_t, u_t, token_idx, u_all, W_a_w, W_a_b, W_g_w, W_g_b, W_f_w, W_f_b):
    # token_idx / u_all are unused by the reference math.
    inputs = {"h_t": h_t, "u_t": u_t, "W_a_w": W_a_w, "W_a_b": W_a_b,
              "W_g_w": W_g_w, "W_g_b": W_g_b, "W_f_w": W_f_w, "W_f_b": W_f_b}
    out, _ = run(inputs)
    return out


# revision 5
# speedup vs baseline: 1.3360x; 1.0302x over previous
"""Trainium2 Bass kernel for nn_ContextDrivingForce (dense MLP, 3 fused layers).

Math (per token row, D=896):
    u_proj = u @ W_a.T + b_a
    alpha  = sigmoid(sum(h * u_proj) / sqrt(D))
    u_att  = alpha * u
    g      = sigmoid([h, u_att] @ W_g.T + b_g)
    u_gate = g * u_att
    out    = gelu([h, u_gate, h*u_gate] @ W_f.T + b_f)        (exact erf gelu)

Distribution: data-parallel over tokens across 8 NeuronCores, weights
replicated. Device tensors feature-major ([D, tokens]); host transposes.

Precision strategy (validated by host-side elementwise simulation of the
exact device dataflow, rel err vs fp32 reference):
  - Layers 1+2 matmuls in fp8e4m3 with DoubleRow (2 k-tiles per MM issue).
    Their output error is damped by the sigmoid gates (|sigma'| <= 1/4).
  - Layer 3: the h-part (87% of z3 variance) stays bf16; the ug/hu parts
    optionally fp8+DoubleRow ("cand2").
  - Elementwise (DVE) chain uses clean bf16 copies of u/h/ua/ug so fp8
    quantization error does not compound through the products.
  - Activations carry embedded scale S_ACT=4, weights S_W=64 (fp8 needs
    operands in normal range; all folds are powers of two).

Sigmoids via tanh (sigmoid(x) = (tanh(x/2)+1)/2) so every ACT op uses the
single `gelu_and_others` table set. Factors of 2 folded into host-side
weight scaling:
    ua' := (tanh(logit/2)+1) * u = 2*u_att   -> W_g ua-cols *= 1/2
    ug' := (t2+1) * ua'          = 4*u_gate  -> W_f ug-cols *= 1/4
    hu' := h * ug'               = 4*h*u_gate-> W_f hu-cols *= 1/4
"""

import math
import sys
from contextlib import ExitStack

for _p in ("/root/.axon_site", "/root/.axon_site/_ro/trn_rl_repo"):
    if _p not in sys.path:
        sys.path.append(_p)

import ml_dtypes
import numpy as np

import concourse.bass as bass
import concourse.mybir as mybir
import concourse.tile as tile
from concourse import bacc
from concourse.bass_utils import run_bass_kernel_spmd

P = 128
D = 896
KD = D // P  # 7 feature tiles
N_TOK = 16384
N_CORES = 8
NPC = N_TOK // N_CORES  # 2048 tokens per core

F32 = mybir.dt.float32
BF16 = mybir.dt.bfloat16
F8 = mybir.dt.float8e4
AF = mybir.ActivationFunctionType
ALU = mybir.AluOpType
DR = mybir.MatmulPerfMode.DoubleRow

S_ACT = 4.0     # embedded scale on h/u and all derived activations
S_W = 64.0      # embedded scale on all weights
S1 = S_ACT * S_W


def build_nc(npc=NPC, T=512, l3fp8=True, clean_hu=True, mm_bufs=6, act_bufs=2):
    """Single-core Bass program (runs SPMD on all 8 cores).

    l3fp8: layer-3 ug/hu k-tiles in fp8 DoubleRow (cand2) vs bf16 (cand1).
    clean_hu: hu computed from a bf16 ug copy instead of the fp8 ug tile.
    """
    n_chunks = npc // T
    assert n_chunks * T == npc

    nc = bacc.Bacc()
    # bf16 h/u glued: [P, nch, 2(h|u), KD, T]
    hub_d = nc.declare_dram_parameter("hub", [P, n_chunks, 2, KD, T], BF16,
                                      isOutput=False)
    h8_d = nc.declare_dram_parameter("h8", [P, n_chunks, KD, T], F8, isOutput=False)
    u8_d = nc.declare_dram_parameter("u8", [P, n_chunks, KD, T], F8, isOutput=False)
    wa_d = nc.declare_dram_parameter("wa", [P, KD, D], F8, isOutput=False)
    # wg k-order: [ua-cols (folded 0.5) | h-cols] to match the uah8 tile
    wg_d = nc.declare_dram_parameter("wg", [P, 2 * KD, D], F8, isOutput=False)
    wfh_d = nc.declare_dram_parameter("wfh", [P, KD, D], BF16, isOutput=False)
    l3dt = F8 if l3fp8 else BF16
    wfuh_d = nc.declare_dram_parameter("wfuh", [P, 2 * KD, D], l3dt, isOutput=False)
    bias_d = nc.declare_dram_parameter("biasp", [P, 3 * KD], F32, isOutput=False)
    gT_d = nc.declare_dram_parameter("gT", [D, npc], BF16, isOutput=True)

    inv_sqrt_d = 1.0 / math.sqrt(D)

    with tile.TileContext(nc) as tc, ExitStack() as ctx:
        wp = ctx.enter_context(tc.tile_pool(name="weights", bufs=1))
        hubp = ctx.enter_context(tc.tile_pool(name="hubp", bufs=act_bufs))
        up = ctx.enter_context(tc.tile_pool(name="up", bufs=act_bufs))
        uahp = ctx.enter_context(tc.tile_pool(name="uahp", bufs=act_bufs))
        uabp = ctx.enter_context(tc.tile_pool(name="uabp", bufs=act_bufs))
        ughup = ctx.enter_context(tc.tile_pool(name="ughup", bufs=act_bufs))
        ugbp = ctx.enter_context(tc.tile_pool(name="ugbp", bufs=act_bufs))
        tmpp = ctx.enter_context(tc.tile_pool(name="tmpp", bufs=act_bufs))
        sp = ctx.enter_context(tc.tile_pool(name="small", bufs=3))
        op = ctx.enter_context(tc.tile_pool(name="outp", bufs=3))
        pp = ctx.enter_context(tc.tile_pool(name="psum", bufs=1, space="PSUM"))

        bias_sb = wp.tile([P, 3 * KD], F32, name="biasp")
        nc.sync.dma_start(bias_sb, bias_d[:, :])
        ones_col = wp.tile([P, 1], BF16, name="ones_col")
        nc.vector.memset(ones_col, 1.0)
        ones_row = wp.tile([1, P], BF16, name="ones_row")
        nc.vector.memset(ones_row, 1.0)

        # ---- weight + chunk-0 prelude, staged for fast compute start ----
        wa_sb = wp.tile([P, KD, D], F8, name="wa")
        u8_0 = up.tile([P, KD, T], F8, name="u8_0", tag="u8")
        uah_0 = uahp.tile([P, 2 * KD, T], F8, name="uah0", tag="uah")
        # first DR pair's operands first: L1 can start after ~0.4 MB
        nc.sync.dma_start(wa_sb[:, 0:2], wa_d[:, 0:2])
        nc.sync.dma_start(u8_0[:, 0:2], u8_d[:, 0, 0:2])
        nc.sync.dma_start(wa_sb[:, 2:], wa_d[:, 2:])
        nc.sync.dma_start(u8_0[:, 2:], u8_d[:, 0, 2:])
        nc.sync.dma_start(uah_0[:, KD:], h8_d[:, 0])  # h8 -> uah slots 7..13
        # weight order matches first-use order: wg (L2.0) -> wfh (L3.0 h-part)
        # -> wfuh (L3.0 DR part)
        wg_sb = wp.tile([P, 2 * KD, D], F8, name="wg")
        nc.sync.dma_start(wg_sb, wg_d[:, :])
        wfh_sb = wp.tile([P, KD, D], BF16, name="wfh")
        nc.sync.dma_start(wfh_sb, wfh_d[:, :])
        wfuh_sb = wp.tile([P, 2 * KD, D], l3dt, name="wfuh")
        nc.sync.dma_start(wfuh_sb, wfuh_d[:, :])
        # bf16 h|u rides the GPSIMD ring so it never blocks the fp8 stream;
        # split so the first h k-tiles land early for the tmp STTs
        hub_0 = hubp.tile([P, 2, KD, T], BF16, name="hub0", tag="hub")
        nc.gpsimd.dma_start(hub_0[:, 0, :4], hub_d[:, 0, 0, :4])
        nc.gpsimd.dma_start(hub_0[:, 0, 4:], hub_d[:, 0, 0, 4:])
        nc.gpsimd.dma_start(hub_0[:, 1], hub_d[:, 0, 1])

        def load_chunk(c):
            hub = hubp.tile([P, 2, KD, T], BF16, name=f"hub{c}", tag="hub")
            nc.gpsimd.dma_start(hub, hub_d[:, c])
            u8 = up.tile([P, KD, T], F8, name=f"u8_{c}", tag="u8")
            nc.sync.dma_start(u8, u8_d[:, c])
            uah = uahp.tile([P, 2 * KD, T], F8, name=f"uah{c}", tag="uah")
            nc.sync.dma_start(uah[:, KD:], h8_d[:, c])
            return hub, u8, uah

        M_GROUPS = [list(range(0, 4)), list(range(4, KD))]

        def emit_l1(c, u8, h_bf, tmp):
            """L1 m-major: per m-tile 3 DR pairs + odd k, then the fused
            (psum+bias)*h DVE op. Low PSUM footprint (2 live) so it can be
            interleaved between the previous chunk's reduce and L2."""
            for m in range(KD):
                ps = pp.tile([P, T], F32, name=f"ps1_{c}_{m}", tag="mm",
                             bufs=mm_bufs)
                for k in range(0, KD - 1, 2):
                    nc.tensor.matmul(ps, lhsT=wa_sb[:, k:k + 2, m * P:(m + 1) * P],
                                     rhs=u8[:, k:k + 2, :], start=(k == 0),
                                     stop=False, perf_mode=DR)
                nc.tensor.matmul(ps, lhsT=wa_sb[:, KD - 1, m * P:(m + 1) * P],
                                 rhs=u8[:, KD - 1, :], start=False, stop=True)
                nc.vector.scalar_tensor_tensor(
                    out=tmp[:, m, :], in0=ps, scalar=bias_sb[:, m:m + 1],
                    in1=h_bf[:, m, :], op0=ALU.add, op1=ALU.mult)

        # chunk 0 L1 up front; chunks c+1 are interleaved into section c below
        state = {}
        state[0] = (hub_0, u8_0, uah_0,
                    tmpp.tile([P, KD, T], BF16, name="tmp0", tag="tmp"))
        emit_l1(0, u8_0, hub_0[:, 0], state[0][3])

        for c in range(n_chunks):
            cs = bass.ds(c * T, T)
            hub, u8, uah, tmp = state.pop(c)
            h_bf = hub[:, 0]   # [P, KD, T] bf16, = S_ACT*h
            u_bf = hub[:, 1]

            # ---- logit reduce over the KD tmp tiles
            red = pp.tile([1, T], F32, name=f"red{c}", tag="red", bufs=1)
            for m in range(KD):
                nc.tensor.matmul(red, lhsT=ones_col, rhs=tmp[:, m, :],
                                 start=(m == 0), stop=(m == KD - 1))

            # alpha' = tanh(logit/2) = 2*sigmoid(logit)-1; logit scale S1*S_ACT
            alpha = sp.tile([1, T], BF16, name=f"al{c}", tag="alpha", bufs=2)
            nc.scalar.activation(alpha, red, AF.Tanh,
                                 scale=inv_sqrt_d * 0.5 / (S1 * S_ACT))
            ab = pp.tile([P, T], F32, name=f"ab{c}", tag="ab", bufs=1)
            nc.tensor.matmul(ab, lhsT=ones_row, rhs=alpha, start=True, stop=True)

            # ua' = (alpha'+1)*u: fp8 (L2 rhs) first so L2 can start ASAP
            for k in range(KD):
                nc.vector.scalar_tensor_tensor(
                    out=uah[:, k, :], in0=ab, scalar=1.0, in1=u8[:, k, :],
                    op0=ALU.add, op1=ALU.mult)

            # next chunk's L1 fills the PE while DVE produces ua / ACT runs
            if c + 1 < n_chunks:
                nhub, nu8, nuah = load_chunk(c + 1)
                ntmp = tmpp.tile([P, KD, T], BF16, name=f"tmp{c + 1}", tag="tmp")
                state[c + 1] = (nhub, nu8, nuah, ntmp)
                emit_l1(c + 1, nu8, nhub[:, 0], ntmp)

            # clean bf16 ua copy (feeds the ug/hu DVE products)
            ua_bf = uabp.tile([P, KD, T], BF16, name=f"uab{c}", tag="uab")
            for k in range(KD):
                nc.vector.scalar_tensor_tensor(
                    out=ua_bf[:, k, :], in0=ab, scalar=1.0, in1=u_bf[:, k, :],
                    op0=ALU.add, op1=ALU.mult)

            # ---- layer 2: z2 = [ua', h] @ wg (7 DR pairs over uah)
            ughu = ughup.tile([P, 2 * KD, T], l3dt, name=f"ughu{c}", tag="ughu")
            ug_bf = ugbp.tile([P, KD, T], BF16, name=f"ugb{c}", tag="ugb") \
                if (l3fp8 and clean_hu) else None
            for grp in M_GROUPS:
                pss = {m: pp.tile([P, T], F32, name=f"ps2_{c}_{m}", tag="mm",
                                  bufs=mm_bufs) for m in grp}
                for k in range(0, 2 * KD, 2):
                    for m in grp:
                        nc.tensor.matmul(pss[m], lhsT=wg_sb[:, k:k + 2, m * P:(m + 1) * P],
                                         rhs=uah[:, k:k + 2, :], start=(k == 0),
                                         stop=(k == 2 * KD - 2), perf_mode=DR)
                for m in grp:
                    t2 = sp.tile([P, T], BF16, name=f"t2_{c}_{m}", tag="t2")
                    nc.scalar.activation(t2, pss[m], AF.Tanh,
                                         bias=bias_sb[:, KD + m:KD + m + 1],
                                         scale=0.5 / S1)
                    # ug' = (t2+1)*ua'  (fp8 for L3 rhs; bf16 copy for hu)
                    nc.vector.scalar_tensor_tensor(
                        out=ughu[:, m, :], in0=t2, scalar=1.0,
                        in1=ua_bf[:, m, :], op0=ALU.add, op1=ALU.mult)
                    if ug_bf is not None:
                        nc.vector.scalar_tensor_tensor(
                            out=ug_bf[:, m, :], in0=t2, scalar=1.0,
                            in1=ua_bf[:, m, :], op0=ALU.add, op1=ALU.mult)
                    # hu' = h*ug' = (h_bf * 1/S_ACT) * ug
                    ug_src = ug_bf if ug_bf is not None else ughu
                    nc.vector.scalar_tensor_tensor(
                        out=ughu[:, KD + m, :], in0=h_bf[:, m, :],
                        scalar=1.0 / S_ACT, in1=ug_src[:, m, :],
                        op0=ALU.mult, op1=ALU.mult)

            # ---- layer 3: out = gelu([h]bf16 + [ug', hu']fp8-DR + b_f)
            for m in range(KD):
                ps = pp.tile([P, T], F32, name=f"ps3_{c}_{m}", tag="mm", bufs=mm_bufs)
                for k in range(KD):
                    nc.tensor.matmul(ps, lhsT=wfh_sb[:, k, m * P:(m + 1) * P],
                                     rhs=h_bf[:, k, :], start=(k == 0), stop=False)
                if l3fp8:
                    for k in range(0, 2 * KD, 2):
                        nc.tensor.matmul(ps, lhsT=wfuh_sb[:, k:k + 2, m * P:(m + 1) * P],
                                         rhs=ughu[:, k:k + 2, :], start=False,
                                         stop=(k == 2 * KD - 2), perf_mode=DR)
                else:
                    for k in range(2 * KD):
                        nc.tensor.matmul(ps, lhsT=wfuh_sb[:, k, m * P:(m + 1) * P],
                                         rhs=ughu[:, k, :], start=False,
                                         stop=(k == 2 * KD - 1))
                outp = op.tile([P, T], BF16, name=f"o{c}_{m}", tag="out")
                nc.scalar.activation(outp, ps, AF.Gelu,
                                     bias=bias_sb[:, 2 * KD + m:2 * KD + m + 1],
                                     scale=1.0 / S1)
                # output stores ride the ACT HWDGE ring
                nc.scalar.dma_start(gT_d[m * P:(m + 1) * P, cs], outp)
    nc.compile()
    return nc


def prep_inputs(h_t, u_t, W_a_w, W_a_b, W_g_w, W_g_b, W_f_w, W_f_b,
                npc=NPC, T=512, l3fp8=True):
    """Host-side: transpose to feature-major, fold scales, quantize, shard."""
    f8 = ml_dtypes.float8_e4m3
    bf16 = ml_dtypes.bfloat16

    h = np.asarray(h_t, np.float32)
    u = np.asarray(u_t, np.float32)
    Wa = np.asarray(W_a_w, np.float32)
    Wg = np.asarray(W_g_w, np.float32)
    Wf = np.asarray(W_f_w, np.float32)
    ba = np.asarray(W_a_b, np.float32)
    bg = np.asarray(W_g_b, np.float32)
    bf = np.asarray(W_f_b, np.float32)

    waT = S_W * Wa.T
    # k-order [ua-cols (x0.5) | h-cols] to match the uah8 tile layout
    wgT = S_W * np.concatenate([Wg[:, D:] * 0.5, Wg[:, :D]], axis=1).T
    wfhT = S_W * Wf[:, :D].T
    wfuhT = S_W * np.concatenate([Wf[:, D:2 * D] * 0.25, Wf[:, 2 * D:] * 0.25],
                                 axis=1).T

    def wpack(w, dt):  # [K_in, D_out] -> [128, K_in/128, D_out]
        return np.ascontiguousarray(
            w.reshape(-1, P, D).transpose(1, 0, 2)).astype(dt)

    wa_p = wpack(waT, f8)
    wg_p = wpack(wgT, f8)
    wfh_p = wpack(wfhT, bf16)
    wfuh_p = wpack(wfuhT, f8 if l3fp8 else bf16)
    biasp = np.ascontiguousarray(
        np.concatenate([S1 * ba, 0.5 * bg, bf]).reshape(3 * KD, P).T
    ).astype(np.float32)

    nch = npc // T

    def xpack(x, i, dt):  # x [N, D] -> [P, nch, KD, T] for core i
        blk = x[i * npc:(i + 1) * npc]                    # [npc, D]
        blk = blk.reshape(nch, T, KD, P)                  # [c, t, k, p]
        return np.ascontiguousarray(blk.transpose(3, 0, 2, 1)).astype(dt)

    hs = S_ACT * h
    us = S_ACT * u
    n_cores = h.shape[0] // npc
    in_maps = []
    for i in range(n_cores):
        hp8 = xpack(hs, i, f8)
        up8 = xpack(us, i, f8)
        hub = np.stack([xpack(hs, i, bf16), xpack(us, i, bf16)], axis=2)
        # hub shape [P, nch, 2, KD, T]
        in_maps.append({
            "hub": np.ascontiguousarray(hub),
            "h8": hp8, "u8": up8,
            "wa": wa_p, "wg": wg_p, "wfh": wfh_p, "wfuh": wfuh_p,
            "biasp": biasp,
        })
    return in_maps


_NC_CACHE = {}


def _get_nc(npc=NPC, T=512, l3fp8=True, clean_hu=True):
    key = (npc, T, l3fp8, clean_hu)
    if key not in _NC_CACHE:
        _NC_CACHE[key] = build_nc(npc=npc, T=T, l3fp8=l3fp8, clean_hu=clean_hu)
    return _NC_CACHE[key]


def run(inputs, npc=NPC, T=512, l3fp8=True, clean_hu=True, trace=False, **kw):
    """Run the SPMD kernel; returns (full fp32 [N,D] output, BassKernelResults)."""
    nc = _get_nc(npc=npc, T=T, l3fp8=l3fp8, clean_hu=clean_hu)
    in_maps = prep_inputs(
        inputs["h_t"], inputs["u_t"], inputs["W_a_w"], inputs["W_a_b"],
        inputs["W_g_w"], inputs["W_g_b"], inputs["W_f_w"], inputs["W_f_b"],
        npc=npc, T=T, l3fp8=l3fp8)
    res = run_bass_kernel_spmd(nc, in_maps, list(range(len(in_maps))),
                               trace=trace, **kw)
    out = np.concatenate(
        [np.asarray(r["gT"]).astype(np.float32).T for r in res.results], axis=0)
    return out, res


def kernel(h_t, u_t, token_idx, u_all, W_a_w, W_a_b, W_g_w, W_g_b, W_f_w, W_f_b):
    # token_idx / u_all are unused by the reference math.
    inputs = {"h_t": h_t, "u_t": u_t, "W_a_w": W_a_w, "W_a_b": W_a_b,
              "W_g_w": W_g_w, "W_g_b": W_g_b, "W_f_w": W_f_w, "W_f_b": W_f_b}
    out, _ = run(inputs)
    return out


# revision 7
# speedup vs baseline: 1.5496x; 1.1599x over previous
"""Trainium2 Bass kernel for nn_ContextDrivingForce (dense MLP, 3 fused layers).

Math (per token row, D=896):
    u_proj = u @ W_a.T + b_a
    alpha  = sigmoid(sum(h * u_proj) / sqrt(D))
    u_att  = alpha * u
    g      = sigmoid([h, u_att] @ W_g.T + b_g)
    u_gate = g * u_att
    out    = gelu([h, u_gate, h*u_gate] @ W_f.T + b_f)        (exact erf gelu)

Distribution: data-parallel over tokens across 8 NeuronCores, weights
replicated. Device tensors feature-major ([D, tokens]); host transposes.

Precision strategy (validated by a host-side elementwise simulation of the
exact device dataflow; sim matched HW to 5 digits on previous revisions):
  - Layers 1+2 matmuls fp8e4m3 with DoubleRow (2 k-tiles per MM issue).
    Their output error is damped by the sigmoid gates (|sigma'| <= 1/4).
  - Layer 3: the h-part (87% of z3 variance) stays bf16; the ug/hu parts
    run fp8+DoubleRow.
  - The DVE product chain uses clean bf16 copies (u_bf -> ua_bf -> ug_bf)
    so fp8 rounding does not compound through the products.
  - Activations carry embedded scale S_ACT=4, weights S_W=64 (keeps fp8
    operands in the normal range; all folds are powers of two).

Schedule: chunks of T=512 tokens, software-pipelined so the PE never waits
on the logit -> tanh -> broadcast -> DVE chain: chunk c+1's layer-1 matmuls
are emitted between chunk c's broadcast and layer 2. All input DMA rides
the sync HWDGE ring in hand-tuned arrival order (the GPSIMD ring is a slow
software-dynamic queue -- do not use it); output stores ride the ACT ring.

Sigmoids via tanh (sigmoid(x) = (tanh(x/2)+1)/2) so every ACT op uses the
single `gelu_and_others` table set; factors of 2 are folded into host-side
weight scaling (W_g ua-cols x0.5, W_f ug/hu-cols x0.25).
"""

import math
import sys
from contextlib import ExitStack

for _p in ("/root/.axon_site", "/root/.axon_site/_ro/trn_rl_repo"):
    if _p not in sys.path:
        sys.path.append(_p)

import ml_dtypes
import numpy as np

import concourse.bass as bass
import concourse.mybir as mybir
import concourse.tile as tile
from concourse import bacc
from concourse.bass_utils import run_bass_kernel_spmd

P = 128
D = 896
KD = D // P  # 7 feature tiles
N_TOK = 16384
N_CORES = 8
NPC = N_TOK // N_CORES  # 2048 tokens per core

F32 = mybir.dt.float32
BF16 = mybir.dt.bfloat16
F8 = mybir.dt.float8e4
AF = mybir.ActivationFunctionType
ALU = mybir.AluOpType
DR = mybir.MatmulPerfMode.DoubleRow

S_ACT = 4.0     # embedded scale on h/u and all derived activations
S_W = 64.0      # embedded scale on all weights
S1 = S_ACT * S_W


def build_nc(npc=NPC, T=512, l3fp8=True, mm_bufs=6, act_bufs=2):
    n_chunks = npc // T
    assert n_chunks * T == npc

    nc = bacc.Bacc()
    hbf_d = nc.declare_dram_parameter("hbf", [P, n_chunks, KD, T], BF16,
                                      isOutput=False)
    ubf_d = nc.declare_dram_parameter("ubf", [P, n_chunks, KD, T], BF16,
                                      isOutput=False)
    h8_d = nc.declare_dram_parameter("h8", [P, n_chunks, KD, T], F8, isOutput=False)
    u8_d = nc.declare_dram_parameter("u8", [P, n_chunks, KD, T], F8, isOutput=False)
    wa_d = nc.declare_dram_parameter("wa", [P, KD, D], F8, isOutput=False)
    # wg k-order: [ua-cols (folded 0.5) | h-cols] to match the uah tile
    wg_d = nc.declare_dram_parameter("wg", [P, 2 * KD, D], F8, isOutput=False)
    wfh_d = nc.declare_dram_parameter("wfh", [P, KD, D], BF16, isOutput=False)
    l3dt = F8 if l3fp8 else BF16
    wfuh_d = nc.declare_dram_parameter("wfuh", [P, 2 * KD, D], l3dt, isOutput=False)
    bias_d = nc.declare_dram_parameter("biasp", [P, 3 * KD], F32, isOutput=False)
    gT_d = nc.declare_dram_parameter("gT", [D, npc], BF16, isOutput=True)

    inv_sqrt_d = 1.0 / math.sqrt(D)

    with tile.TileContext(nc) as tc, ExitStack() as ctx:
        wp = ctx.enter_context(tc.tile_pool(name="weights", bufs=1))
        hbp = ctx.enter_context(tc.tile_pool(name="hbp", bufs=act_bufs))
        ubp = ctx.enter_context(tc.tile_pool(name="ubp", bufs=act_bufs))
        up = ctx.enter_context(tc.tile_pool(name="up", bufs=3))
        uahp = ctx.enter_context(tc.tile_pool(name="uahp", bufs=3))
        uabp = ctx.enter_context(tc.tile_pool(name="uabp", bufs=act_bufs))
        ughup = ctx.enter_context(tc.tile_pool(name="ughup", bufs=act_bufs))
        ugbp = ctx.enter_context(tc.tile_pool(name="ugbp", bufs=act_bufs))
        tmpp = ctx.enter_context(tc.tile_pool(name="tmpp", bufs=act_bufs))
        sp = ctx.enter_context(tc.tile_pool(name="small", bufs=3))
        op = ctx.enter_context(tc.tile_pool(name="outp", bufs=3))
        pp = ctx.enter_context(tc.tile_pool(name="psum", bufs=1, space="PSUM"))

        bias_sb = wp.tile([P, 3 * KD], F32, name="biasp")
        ones_col = wp.tile([P, 1], BF16, name="ones_col")
        nc.vector.memset(ones_col, 1.0)
        ones_row = wp.tile([1, P], BF16, name="ones_row")
        nc.vector.memset(ones_row, 1.0)

        # ---- SBUF weight tiles
        wa_sb = wp.tile([P, KD, D], F8, name="wa")
        wg_sb = wp.tile([P, 2 * KD, D], F8, name="wg")
        wfh_sb = wp.tile([P, KD, D], BF16, name="wfh")
        wfuh_sb = wp.tile([P, 2 * KD, D], l3dt, name="wfuh")

        # per-chunk input tiles, allocated lazily
        tiles = {}

        def alloc_chunk(c):
            tiles[c] = {
                "u8": up.tile([P, KD, T], F8, name=f"u8_{c}", tag="u8"),
                "uah": uahp.tile([P, 2 * KD, T], F8, name=f"uah{c}", tag="uah"),
                "hbf": hbp.tile([P, KD, T], BF16, name=f"hbf{c}", tag="hbf"),
                "ubf": ubp.tile([P, KD, T], BF16, name=f"ubf{c}", tag="ubf"),
                "tmp": tmpp.tile([P, KD, T], BF16, name=f"tmp{c}", tag="tmp"),
            }
            return tiles[c]

        # ---- prelude DMA, hand-ordered for earliest compute start.
        # sync ring is FIFO: arrival order == trigger order.
        alloc_chunk(0)
        alloc_chunk(1)
        nc.sync.dma_start(bias_sb, bias_d[:, :])
        nc.sync.dma_start(wa_sb[:, 0:2], wa_d[:, 0:2])
        nc.sync.dma_start(tiles[0]["u8"][:, 0:2], u8_d[:, 0, 0:2])
        nc.sync.dma_start(wa_sb[:, 2:], wa_d[:, 2:])
        nc.sync.dma_start(tiles[0]["u8"][:, 2:], u8_d[:, 0, 2:])
        nc.sync.dma_start(tiles[0]["uah"][:, KD:], h8_d[:, 0])
        nc.sync.dma_start(tiles[1]["u8"], u8_d[:, 1])
        nc.sync.dma_start(tiles[1]["uah"][:, KD:], h8_d[:, 1])
        nc.sync.dma_start(wg_sb, wg_d[:, :])
        nc.sync.dma_start(tiles[0]["ubf"], ubf_d[:, 0])
        nc.sync.dma_start(tiles[0]["hbf"], hbf_d[:, 0])
        nc.sync.dma_start(wfh_sb, wfh_d[:, :])
        nc.sync.dma_start(wfuh_sb, wfuh_d[:, :])

        def emit_l1(c):
            """L1 m-major: per m-tile 3 DR pairs + odd k into one psum, then
            the fused (psum + S1*b_a) * h8 DVE op, then the DVE reduce tree.
            tmp uses the fp8 h copy so the bf16 h is off the critical path."""
            t = tiles[c]
            u8, uah, tmp = t["u8"], t["uah"], t["tmp"]
            for m in range(KD):
                ps = pp.tile([P, T], F32, name=f"ps1_{c}_{m}", tag="mm",
                             bufs=mm_bufs)
                for k in range(0, KD - 1, 2):
                    nc.tensor.matmul(ps, lhsT=wa_sb[:, k:k + 2, m * P:(m + 1) * P],
                                     rhs=u8[:, k:k + 2, :], start=(k == 0),
                                     stop=False, perf_mode=DR)
                nc.tensor.matmul(ps, lhsT=wa_sb[:, KD - 1, m * P:(m + 1) * P],
                                 rhs=u8[:, KD - 1, :], start=False, stop=True)
                nc.vector.scalar_tensor_tensor(
                    out=tmp[:, m, :], in0=ps, scalar=bias_sb[:, m:m + 1],
                    in1=uah[:, KD + m, :], op0=ALU.add, op1=ALU.mult)
            # partition-reduce prep: 7 -> 1 tile on DVE (saves 6 PE matmuls);
            # runs one section ahead of its reduce matmul, so zero latency.
            s0 = sp.tile([P, T], BF16, name=f"s0_{c}", tag="tree", bufs=10)
            s1 = sp.tile([P, T], BF16, name=f"s1_{c}", tag="tree", bufs=10)
            s2 = sp.tile([P, T], BF16, name=f"s2_{c}", tag="tree", bufs=10)
            s3 = sp.tile([P, T], BF16, name=f"s3_{c}", tag="tree", bufs=10)
            s4 = sp.tile([P, T], BF16, name=f"s4_{c}", tag="tree", bufs=10)
            nc.vector.tensor_add(out=s0, in0=tmp[:, 0, :], in1=tmp[:, 1, :])
            nc.vector.tensor_add(out=s1, in0=tmp[:, 2, :], in1=tmp[:, 3, :])
            nc.vector.tensor_add(out=s2, in0=tmp[:, 4, :], in1=tmp[:, 5, :])
            nc.vector.tensor_add(out=s3, in0=s0, in1=s1)
            nc.vector.tensor_add(out=s4, in0=s2, in1=tmp[:, 6, :])
            rsum = sp.tile([P, T], BF16, name=f"rs_{c}", tag="rsum", bufs=2)
            nc.vector.tensor_add(out=rsum, in0=s3, in1=s4)
            t["rsum"] = rsum

        emit_l1(0)

        for c in range(n_chunks):
            cs = bass.ds(c * T, T)
            t = tiles.pop(c)
            u8, uah, hbf, ubf = t["u8"], t["uah"], t["hbf"], t["ubf"]

            # ---- logit: single partition-reduce matmul of the DVE tree sum
            red = pp.tile([1, T], F32, name=f"red{c}", tag="red", bufs=1)
            nc.tensor.matmul(red, lhsT=ones_col, rhs=t["rsum"], start=True,
                             stop=True)
            # alpha' = tanh(logit/2); logit embedded scale S1*S_ACT
            alpha = sp.tile([1, T], BF16, name=f"al{c}", tag="alpha", bufs=2)
            nc.scalar.activation(alpha, red, AF.Tanh,
                                 scale=inv_sqrt_d * 0.5 / (S1 * S_ACT))
            ab = pp.tile([P, T], F32, name=f"ab{c}", tag="ab", bufs=1)
            nc.tensor.matmul(ab, lhsT=ones_row, rhs=alpha, start=True, stop=True)

            # ua' = (alpha'+1)*u -> fp8 (L2 rhs) first so L2 can start ASAP
            for k in range(KD):
                nc.vector.scalar_tensor_tensor(
                    out=uah[:, k, :], in0=ab, scalar=1.0, in1=u8[:, k, :],
                    op0=ALU.add, op1=ALU.mult)

            # next chunk's L1 fills the PE while ACT/DVE produce alpha & ua;
            # chunk c+2's fp8 inputs + c+1's bf16 inputs stream behind it
            if c + 1 < n_chunks:
                if c + 2 < n_chunks:
                    nt = alloc_chunk(c + 2)
                    nc.sync.dma_start(nt["u8"], u8_d[:, c + 2])
                    nc.sync.dma_start(nt["uah"][:, KD:], h8_d[:, c + 2])
                nc.sync.dma_start(tiles[c + 1]["ubf"], ubf_d[:, c + 1])
                nc.sync.dma_start(tiles[c + 1]["hbf"], hbf_d[:, c + 1])
                emit_l1(c + 1)

            # clean bf16 ua copy (feeds the ug/hu DVE products)
            ua_bf = uabp.tile([P, KD, T], BF16, name=f"uab{c}", tag="uab")
            for k in range(KD):
                nc.vector.scalar_tensor_tensor(
                    out=ua_bf[:, k, :], in0=ab, scalar=1.0, in1=ubf[:, k, :],
                    op0=ALU.add, op1=ALU.mult)

            # ---- layer 2: z2 = [ua', h] @ wg (7 DR pairs over uah)
            M_GROUPS = [list(range(0, 4)), list(range(4, KD))]
            t2s = {}
            for grp in M_GROUPS:
                pss = {m: pp.tile([P, T], F32, name=f"ps2_{c}_{m}", tag="mm",
                                  bufs=mm_bufs) for m in grp}
                for k in range(0, 2 * KD, 2):
                    for m in grp:
                        nc.tensor.matmul(pss[m], lhsT=wg_sb[:, k:k + 2, m * P:(m + 1) * P],
                                         rhs=uah[:, k:k + 2, :], start=(k == 0),
                                         stop=(k == 2 * KD - 2), perf_mode=DR)
                for m in grp:
                    t2 = sp.tile([P, T], BF16, name=f"t2_{c}_{m}", tag="t2",
                                 bufs=KD)
                    nc.scalar.activation(t2, pss[m], AF.Tanh,
                                         bias=bias_sb[:, KD + m:KD + m + 1],
                                         scale=0.5 / S1)
                    t2s[m] = t2

            # ug' = (t2+1)*ua' fp8 first (flat order so L3's DR pairs can
            # start before the hu slots finish), then bf16 copies, then hu
            ughu = ughup.tile([P, 2 * KD, T], l3dt, name=f"ughu{c}", tag="ughu")
            ug_bf = ugbp.tile([P, KD, T], BF16, name=f"ugb{c}", tag="ugb")
            for m in range(KD):
                nc.vector.scalar_tensor_tensor(
                    out=ughu[:, m, :], in0=t2s[m], scalar=1.0,
                    in1=ua_bf[:, m, :], op0=ALU.add, op1=ALU.mult)
            for m in range(KD):
                nc.vector.scalar_tensor_tensor(
                    out=ug_bf[:, m, :], in0=t2s[m], scalar=1.0,
                    in1=ua_bf[:, m, :], op0=ALU.add, op1=ALU.mult)
            for m in range(KD):
                # hu' = h*ug' = (h_bf * 1/S_ACT) * ug_bf
                nc.vector.scalar_tensor_tensor(
                    out=ughu[:, KD + m, :], in0=hbf[:, m, :],
                    scalar=1.0 / S_ACT, in1=ug_bf[:, m, :],
                    op0=ALU.mult, op1=ALU.mult)

            # ---- layer 3: out = gelu([h]bf16 + [ug', hu']fp8-DR + b_f)
            for m in range(KD):
                ps = pp.tile([P, T], F32, name=f"ps3_{c}_{m}", tag="mm", bufs=mm_bufs)
                for k in range(KD):
                    nc.tensor.matmul(ps, lhsT=wfh_sb[:, k, m * P:(m + 1) * P],
                                     rhs=hbf[:, k, :], start=(k == 0), stop=False)
                if l3fp8:
                    for k in range(0, 2 * KD, 2):
                        nc.tensor.matmul(ps, lhsT=wfuh_sb[:, k:k + 2, m * P:(m + 1) * P],
                                         rhs=ughu[:, k:k + 2, :], start=False,
                                         stop=(k == 2 * KD - 2), perf_mode=DR)
                else:
                    for k in range(2 * KD):
                        nc.tensor.matmul(ps, lhsT=wfuh_sb[:, k, m * P:(m + 1) * P],
                                         rhs=ughu[:, k, :], start=False,
                                         stop=(k == 2 * KD - 1))
                outp = op.tile([P, T], BF16, name=f"o{c}_{m}", tag="out")
                nc.scalar.activation(outp, ps, AF.Gelu,
                                     bias=bias_sb[:, 2 * KD + m:2 * KD + m + 1],
                                     scale=1.0 / S1)
                nc.scalar.dma_start(gT_d[m * P:(m + 1) * P, cs], outp)
    nc.compile()
    return nc


def prep_inputs(h_t, u_t, W_a_w, W_a_b, W_g_w, W_g_b, W_f_w, W_f_b,
                npc=NPC, T=512, l3fp8=True):
    """Host-side: transpose to feature-major, fold scales, quantize, shard."""
    f8 = ml_dtypes.float8_e4m3
    bf16 = ml_dtypes.bfloat16

    h = np.asarray(h_t, np.float32)
    u = np.asarray(u_t, np.float32)
    Wa = np.asarray(W_a_w, np.float32)
    Wg = np.asarray(W_g_w, np.float32)
    Wf = np.asarray(W_f_w, np.float32)
    ba = np.asarray(W_a_b, np.float32)
    bg = np.asarray(W_g_b, np.float32)
    bf = np.asarray(W_f_b, np.float32)

    waT = S_W * Wa.T
    wgT = S_W * np.concatenate([Wg[:, D:] * 0.5, Wg[:, :D]], axis=1).T
    wfhT = S_W * Wf[:, :D].T
    wfuhT = S_W * np.concatenate([Wf[:, D:2 * D] * 0.25, Wf[:, 2 * D:] * 0.25],
                                 axis=1).T

    def wpack(w, dt):  # [K_in, D_out] -> [128, K_in/128, D_out]
        return np.ascontiguousarray(
            w.reshape(-1, P, D).transpose(1, 0, 2)).astype(dt)

    wa_p = wpack(waT, f8)
    wg_p = wpack(wgT, f8)
    wfh_p = wpack(wfhT, bf16)
    wfuh_p = wpack(wfuhT, f8 if l3fp8 else bf16)
    biasp = np.ascontiguousarray(
        np.concatenate([S1 * ba, 0.5 * bg, bf]).reshape(3 * KD, P).T
    ).astype(np.float32)

    nch = npc // T

    def xpack(x, i, dt):  # x [N, D] -> [P, nch, KD, T] for core i
        blk = x[i * npc:(i + 1) * npc]                    # [npc, D]
        blk = blk.reshape(nch, T, KD, P)                  # [c, t, k, p]
        return np.ascontiguousarray(blk.transpose(3, 0, 2, 1)).astype(dt)

    hs = S_ACT * h
    us = S_ACT * u
    n_cores = h.shape[0] // npc
    in_maps = []
    for i in range(n_cores):
        in_maps.append({
            "hbf": xpack(hs, i, bf16), "ubf": xpack(us, i, bf16),
            "h8": xpack(hs, i, f8), "u8": xpack(us, i, f8),
            "wa": wa_p, "wg": wg_p, "wfh": wfh_p, "wfuh": wfuh_p,
            "biasp": biasp,
        })
    return in_maps


_NC_CACHE = {}


def _get_nc(npc=NPC, T=512, l3fp8=True):
    key = (npc, T, l3fp8)
    if key not in _NC_CACHE:
        _NC_CACHE[key] = build_nc(npc=npc, T=T, l3fp8=l3fp8)
    return _NC_CACHE[key]


def run(inputs, npc=NPC, T=512, l3fp8=True, clean_hu=True, trace=False, **kw):
    """Run the SPMD kernel; returns (full fp32 [N,D] output, BassKernelResults)."""
    nc = _get_nc(npc=npc, T=T, l3fp8=l3fp8)
    in_maps = prep_inputs(
        inputs["h_t"], inputs["u_t"], inputs["W_a_w"], inputs["W_a_b"],
        inputs["W_g_w"], inputs["W_g_b"], inputs["W_f_w"], inputs["W_f_b"],
        npc=npc, T=T, l3fp8=l3fp8)
    res = run_bass_kernel_spmd(nc, in_maps, list(range(len(in_maps))),
                               trace=trace, **kw)
    out = np.concatenate(
        [np.asarray(r["gT"]).astype(np.float32).T for r in res.results], axis=0)
    return out, res


def kernel(h_t, u_t, token_idx, u_all, W_a_w, W_a_b, W_g_w, W_g_b, W_f_w, W_f_b):
    # token_idx / u_all are unused by the reference math.
    inputs = {"h_t": h_t, "u_t": u_t, "W_a_w": W_a_w, "W_a_b": W_a_b,
              "W_g_w": W_g_w, "W_g_b": W_g_b, "W_f_w": W_f_w, "W_f_b": W_f_b}
    out, _ = run(inputs)
    return out
